# revision 13
# baseline (speedup 1.0000x reference)
"""Trainium2 Bass kernel for nn_ComplexUnitaryGCN (2-layer complex unitary GCN,
circulant 16-regular graph, N=100000 nodes, D=128 dims, 8 NeuronCores).

Strategy (self-contained; shapes/sharding hardcoded):
  - Shard nodes across the 8 cores (12500 rows each) with replicated halos
    (host-side wraparound slicing) - no device-to-device communication.
  - Device works in feature-major layout: xT slab [128 dims, L+36 nodes];
    host pre-transposes the input slab and post-transposes the output.
  - The star-graph evolution row has equal weights on all 16 leaves, so the
    per-node neighbor aggregation is w0*h + wbar*(17-window sum along the
    node axis). Window sums are ONE fused DVE op each: tensor_tensor_scan
    computes state = (h[c+8] + state) - h[c-9] (fp32 state, fp16 data),
    chained across tiles via initial=prev[:, -1:].
  - All slabs/GEMM operands are fp16 (PSUM accumulates fp32).
  - Layer-2 GEMM folds the layer-1 aggregation scalars into pre-scaled
    complex weight matrices (host-side), accumulating 8 fp16 matmuls into
    PSUM per output component. The b2 bias is folded into the ns1 scan
    initial via gamma = Wb^-1 b2p, so both crelu components collapse into a
    single wide bias-free Relu on a 2-bank PSUM tile.
  - Layer-1 / window-scan / stage-2 loops are interleaved in one pass with
    a 1-2 step stagger (plus a delayed final-combine sub-stage) so PE, ACT
    and DVE all stream without head-of-line stalls; GPSIMD is avoided
    entirely (Pool TensorTensor measures ~1.9us/op on HW).
"""

import numpy as np

# ---------------------------------------------------------------- constants
N = 100000
D = 128
NCORES = 8
L = N // NCORES           # 12500 nodes per core
HL, HR = 18, 18           # left/right slab halo
LH = L + HL + HR          # 12536 slab columns
CHUNK = 492               # stage-2 output chunk (CHUNK+18 <= 512 PSUM bank)
L1_CHUNK = 512            # layer-1 GEMM chunk
L1_PAIR = 1024            # layer-1 ACT width (2 GEMM chunks, 2 PSUM banks)
PW = 1024                 # window-1 diff/scan piece width
NS = L + 18               # ns1 slab cols, col j = center (HL-10+j)
DEG = 16

_PROGRAM = None           # cached compiled program
_VARIANT = "v3"           # 'v3' restructured; 'v2' fp16 pipeline; 'v1' fp32

# v3 tuning knobs (safe defaults; flipped by measurement)
_V3 = dict(
    gp_d=False,       # emit the d' combine on GPSIMD instead of DVE
    gp_out=True,      # emit the out' combine on GPSIMD instead of DVE
    relu_b_every=0,   # 0=off; else every k-th chunk uses the fused STT-relu path
)


# ------------------------------------------------------------- host helpers
def _evolution_row(deg, tr, ti):
    """Replicate reference._evolution_row (jax f32 on CPU when available)."""
    try:
        import jax

        cpu = jax.devices("cpu")[0]
        with jax.default_device(cpu):
            import jax.numpy as jnp

            n = deg + 1
            A = jnp.zeros((n, n), jnp.complex64).at[0, 1:].set(1.0).at[1:, 0].set(1.0)
            t = (jnp.float32(tr) + 1j * jnp.float32(ti)).astype(jnp.complex64)
            G = jax.scipy.linalg.expm(-1j * A * t)
            s = jnp.sqrt(jnp.max(jnp.linalg.eigvalsh(G @ G.conj().T))).astype(
                jnp.complex64
            )
            Lt = G / s
            Rt = jnp.sqrt(jnp.eye(n, dtype=jnp.complex64) - Lt @ (G.conj().T / s))
            return np.asarray(Lt[0] + Rt[0])
    except Exception:
        n = deg + 1
        A = np.zeros((n, n), np.float64)
        A[0, 1:] = 1.0
        A[1:, 0] = 1.0
        t = complex(tr, ti)
        evals, evecs = np.linalg.eigh(A)
        G = (evecs * np.exp(-1j * evals * t)) @ evecs.T
        s = np.sqrt(np.max(np.linalg.eigvalsh(G @ G.conj().T)))
        Lt = G / s
        Rt = np.sqrt(np.eye(n) - Lt @ (G.conj().T / s))
        return (Lt[0] + Rt[0]).astype(np.complex64)


def _fold_weights(ins, w1, w2):
    """Pre-scale/transpose all weights into the device layouts (f32)."""
    W1r, W1i = ins["W1r"], ins["W1i"]
    W2c = ins["W2r"] + 1j * ins["W2i"]
    b2c = ins["b2r"] + 1j * ins["b2i"]
    eb1c = ins["eb1r"] + 1j * ins["eb1i"]
    w0_1, wb_1 = w1[0], w1[1:].mean()
    w0_2, wb_2 = w2[0], w2[1:].mean()
    Wa = (w0_1 - wb_1) * W2c          # layer-2 direct-h1 term
    Wb = wb_1 * W2c                   # layer-2 window-sum term
    b2p = b2c + W2c @ eb1c            # eb1 folded through GEMM2
    c_h2r = (w0_2 - wb_2).real
    c_h2i = -(w0_2 - wb_2).imag
    c_nr = wb_2.real
    c_ni = -wb_2.imag
    c0 = c_h2r
    f32 = np.float32
    # lhsT layout: [K=feat_in partitions, M=feat_out] == numpy transpose of [out,in]
    wl1 = np.concatenate([W1r.T, W1i.T], axis=1).astype(f32)          # [128, 256]
    wg = np.concatenate(
        [
            Wa.real.T, -Wa.imag.T, Wb.real.T, -Wb.imag.T,             # -> gr
            Wa.imag.T, Wa.real.T, Wb.imag.T, Wb.real.T,               # -> gi
        ],
        axis=1,
    ).astype(f32)                                                      # [128, 1024]
    # gamma-fold: ns1 + gamma pushed through the Wb part of GEMM2 reproduces
    # the b2 bias, freeing the stage-2 activation to be a single wide Relu
    use_gamma = False
    g_r = g_i = np.zeros(D)
    try:
        gamma = np.linalg.solve(Wb, b2p)
        if np.all(np.isfinite(gamma)) and np.abs(gamma).max() < 100.0:
            use_gamma = True
            g_r, g_i = gamma.real, gamma.imag
    except Exception:
        pass
    biases = np.stack(
        [
            ins["b1r"], ins["b1i"],
            g_r if use_gamma else b2p.real,
            g_i if use_gamma else b2p.imag,
            ins["eb2r"], np.zeros(D), np.zeros(D), np.zeros(D),
        ],
        axis=1,
    ).astype(f32)                                                      # [128, 8]
    # layer-2 window term: c_nr*W17(h2r) + c_ni*W17(h2i) = q*W17(h2c) with
    # h2c a normalized combine of h2r/h2i (pick larger coeff for conditioning)
    if max(abs(c_nr), abs(c_ni)) == 0.0:
        h2c_on_r, h2c_scale, q = False, 0.0, 0.0
    elif abs(c_nr) >= abs(c_ni):
        h2c_on_r, h2c_scale, q = False, c_ni / c_nr, c_nr   # h2c = h2r + s*h2i
    else:
        h2c_on_r, h2c_scale, q = True, c_nr / c_ni, c_ni    # h2c = s*h2r + h2i
    scalars = dict(
        r1=float(c_h2i / c0), c0=float(c0),
        h2c_on_r=bool(h2c_on_r), h2c_scale=float(h2c_scale), qn=float(q / c0),
        use_gamma=use_gamma,
    )
    return wl1, wg, biases, scalars


def _fold_weights_v3(ins, w1, w2):
    """v3 folding: per-component |scale| pushed into wg so the stage-2
    epilogue is Relu (no scale) + two TT combines; a global sign sigma is
    applied on the host. Returns (wl1, wg, biases, scalars) with extra v3
    keys, or scalars['v3_ok']=False when the structure doesn't apply."""
    wl1, wg, biases, sc = _fold_weights(ins, w1, w2)
    sc = dict(sc)
    sc["v3_ok"] = False
    drop_h2c = abs(sc["h2c_scale"]) < 0.01
    if not (drop_h2c and sc["use_gamma"]):
        return wl1, wg, biases, sc

    W2c = ins["W2r"] + 1j * ins["W2i"]
    w0, wb = w2[0], w2[1:].mean()
    c_h2r = (w0 - wb).real
    c_h2i = -(w0 - wb).imag
    c_nr = wb.real
    c_ni = -wb.imag
    # scan-input component: the one with the larger window coefficient
    wcomp_is_i = bool(sc["h2c_on_r"])      # True -> window input = h2i
    q = c_ni if wcomp_is_i else c_nr
    a_w = c_h2i if wcomp_is_i else c_h2r   # direct coeff of the scan comp
    a_d = c_h2r if wcomp_is_i else c_h2i   # direct coeff of the other comp
    if a_w == 0.0 or a_d == 0.0:
        return wl1, wg, biases, sc
    f_r, f_i = abs(c_h2r), abs(c_h2i)
    wg = wg.copy()
    wg[:, 0 : 4 * D] *= np.float32(f_r)
    wg[:, 4 * D : 8 * D] *= np.float32(f_i)
    sigma = 1.0 if a_d > 0 else -1.0
    sm = sigma * q / abs(a_w)              # coeff of wt~ in out'
    sw = sigma * (1.0 if a_w > 0 else -1.0)  # coeff of h2w~ in out'
    sc.update(
        v3_ok=True, v3_sigma=float(sigma), v3_sm=float(sm), v3_sw=float(sw),
        v3_wcomp_is_i=wcomp_is_i,
    )
    return wl1, wg, biases, sc


def _is_circulant(edge_index):
    """Check edge_index matches the reference's circulant construction."""
    if edge_index.shape != (2, N * DEG // 2):
        return False
    K = DEG // 2
    i = np.arange(N)
    src = np.repeat(i, K)
    dst = ((i[:, None] + np.arange(1, K + 1)[None, :]) % N).reshape(-1)
    return bool(
        np.array_equal(edge_index[0], src) and np.array_equal(edge_index[1], dst)
    )


def _fallback_numpy(ins):
    """Exact reference semantics on host (any edge_index). Slow but correct."""
    x = ins["x"]
    edge_index = ins["edge_index"]
    src, dst = edge_index[0], edge_index[1]
    nodes = np.concatenate([src, dst])
    nbr = np.concatenate([dst, src])
    order = np.lexsort((nbr, nodes))
    deg = nodes.shape[0] // N
    nbrs = nbr[order].reshape(N, deg)
    h = x.astype(np.complex64)

    def crelu(z):
        return (np.maximum(z.real, 0) + 1j * np.maximum(z.imag, 0)).astype(
            np.complex64
        )

    for l in ("1", "2"):
        W = (ins[f"W{l}r"] + 1j * ins[f"W{l}i"]).astype(np.complex64)
        b = (ins[f"b{l}r"] + 1j * ins[f"b{l}i"]).astype(np.complex64)
        h = crelu(h @ W.T + b)
        w = _evolution_row(deg, float(ins[f"t{l}r"]), float(ins[f"t{l}i"]))
        out = w[0] * h
        for k in range(deg):
            out = out + w[1 + k] * h[nbrs[:, k]]
        h = (out + (ins[f"eb{l}r"] + 1j * ins[f"eb{l}i"])).astype(np.complex64)
    return np.ascontiguousarray(h.real.astype(np.float32))


# ------------------------------------------------------------ device program
def _build_program_v3(reps=1):
    """v3: stage-2 epilogue reduced to [wt scan + d' TT + out' TT] per chunk
    (scales folded into wg, global sign fixed up on host), optional GPSIMD
    offload of the TT combines, fused STT-relu path for ACT/DVE balance."""
    import concourse.bacc as bacc
    import concourse.mybir as mybir
    import concourse.tile as tile

    f32 = mybir.dt.float32
    f16 = mybir.dt.float16
    AF = mybir.ActivationFunctionType
    OP = mybir.AluOpType
    AX = mybir.AxisListType
    sc = _build_program.scalars
    assert sc.get("v3_ok")
    sm, sw = sc["v3_sm"], sc["v3_sw"]
    wcomp_is_i = sc["v3_wcomp_is_i"]
    # d' = sm*wt + sw*h2w as one op (TT when |sm|==1, else STT)
    tt_d = abs(abs(sm) - 1.0) < 1e-4
    gp_d, gp_out = _V3["gp_d"], _V3["gp_out"]
    rbe = _V3["relu_b_every"]

    nc = bacc.Bacc("TRN2", target_bir_lowering=False, debug=False)

    xT = nc.dram_tensor("xT", [D, LH], f16, kind="ExternalInput")
    wl1_d = nc.dram_tensor("wl1", [D, 2 * D], f16, kind="ExternalInput")
    wg_d = nc.dram_tensor("wg", [D, 8 * D], f16, kind="ExternalInput")
    bias_d = nc.dram_tensor("biases", [D, 8], f32, kind="ExternalInput")
    outT = nc.dram_tensor("outT", [D, L], f16, kind="ExternalOutput")
    dump = _V3.get("dump")
    if dump:
        dbg_d = nc.dram_tensor("dbg", [D, 8 * 512], f16, kind="ExternalOutput")

    with tile.TileContext(nc) as tc:
        with (
            tc.tile_pool(name="consts", bufs=1) as cpool,
            tc.tile_pool(name="slab", bufs=1) as slab,
            tc.tile_pool(name="xs", bufs=3) as xs,
            tc.tile_pool(name="ps1", bufs=1, space="PSUM") as ps1,
            tc.tile_pool(name="ps2", bufs=2, space="PSUM") as ps2,
            tc.tile_pool(name="st2", bufs=4) as st2,
            tc.tile_pool(name="outp", bufs=4) as outp,
        ):
            wl1 = cpool.tile([D, 2 * D], f16)
            wg = cpool.tile([D, 8 * D], f16)
            bias = cpool.tile([D, 8], f32)
            nc.sync.dma_start(wl1[:], wl1_d[:])
            nc.sync.dma_start(wg[:], wg_d[:])
            nc.sync.dma_start(bias[:], bias_d[:])

            for _rep in range(reps):
                h1r = slab.tile([D, LH], f16, tag="h1r", bufs=2)
                h1i = slab.tile([D, LH], f16, tag="h1i", bufs=2)
                ns1r = slab.tile([D, NS], f16, tag="ns1r")
                ns1i = slab.tile([D, NS], f16, tag="ns1i")
                init1 = slab.tile([D, 4], f32, tag="init1")
                init2 = slab.tile([D, 4], f32, tag="init2")
                state = dict(prev_wt=None, prev_cw=0, a={})

                def dbg_dump(slot, src, w=512):
                    if dump and _rep == 0:
                        nc.sync.dma_start(
                            dbg_d[:, slot * 512 : slot * 512 + w], src
                        )

                def do_l1(k):
                    s = k * L1_PAIR
                    cw2 = min(L1_PAIR, LH - s)
                    xt = xs.tile([D, L1_PAIR], f16)
                    nc.sync.dma_start(xt[:, :cw2], xT[:, s : s + cw2])
                    pr = ps1.tile([D, L1_PAIR], f32, tag="ps1r")
                    pi = ps1.tile([D, L1_PAIR], f32, tag="ps1i")
                    for h0 in range(0, cw2, L1_CHUNK):
                        hw = min(L1_CHUNK, cw2 - h0)
                        nc.tensor.matmul(
                            pr[:, h0 : h0 + hw], wl1[:, 0:D], xt[:, h0 : h0 + hw],
                            start=True, stop=True,
                        )
                        nc.tensor.matmul(
                            pi[:, h0 : h0 + hw], wl1[:, D : 2 * D],
                            xt[:, h0 : h0 + hw], start=True, stop=True,
                        )
                    nc.scalar.activation(
                        h1r[:, s : s + cw2], pr[:, :cw2], AF.Relu, bias=bias[:, 0:1]
                    )
                    nc.scalar.activation(
                        h1i[:, s : s + cw2], pi[:, :cw2], AF.Relu, bias=bias[:, 1:2]
                    )
                    if k == 0:
                        dbg_dump(0, h1r[:, 0:512])
                        dbg_dump(1, h1i[:, 0:512])

                def do_win_init():
                    nc.vector.tensor_reduce(init1[:, 0:1], h1r[:, 0:17], AX.X, OP.add)
                    nc.vector.tensor_reduce(init1[:, 1:2], h1i[:, 0:17], AX.X, OP.add)
                    nc.vector.tensor_reduce(init1[:, 2:3], h1r[:, 1:18], AX.X, OP.add)
                    nc.vector.tensor_reduce(init1[:, 3:4], h1i[:, 1:18], AX.X, OP.add)
                    for c, gcol in ((0, 2), (1, 3), (2, 2), (3, 3)):
                        nc.vector.tensor_scalar(
                            init2[:, c : c + 1], init1[:, c : c + 1], 1.0,
                            bias[:, gcol : gcol + 1], OP.mult, OP.add,
                        )
                    nc.vector.tensor_copy(ns1r[:, 0:1], init2[:, 0:1])
                    nc.vector.tensor_copy(ns1i[:, 0:1], init2[:, 1:2])
                    nc.vector.tensor_copy(ns1r[:, 1:2], init2[:, 2:3])
                    nc.vector.tensor_copy(ns1i[:, 1:2], init2[:, 3:4])

                def do_win(p):
                    s = 2 + p * PW
                    w = min(PW, NS - s)
                    ir = init2[:, 2:3] if p == 0 else ns1r[:, s - 1 : s]
                    ii = init2[:, 3:4] if p == 0 else ns1i[:, s - 1 : s]
                    nc.vector.tensor_tensor_scan(
                        ns1r[:, s : s + w], h1r[:, s + 16 : s + 16 + w],
                        h1r[:, s - 1 : s - 1 + w], ir, OP.add, OP.subtract,
                    )
                    nc.vector.tensor_tensor_scan(
                        ns1i[:, s : s + w], h1i[:, s + 16 : s + 16 + w],
                        h1i[:, s - 1 : s - 1 + w], ii, OP.add, OP.subtract,
                    )
                    if p == 0:
                        dbg_dump(2, ns1r[:, 0:512])
                        dbg_dump(3, ns1i[:, 0:512])

                def do_stage2a(k):
                    a = HL + k * CHUNK
                    cw = min(CHUNK, L - k * CHUNK)
                    ws = a - 10
                    w2n = cw + 18
                    nscol = k * CHUNK
                    path_b = rbe and (k % rbe == 0) and k > 0

                    rhs_list = [
                        h1r[:, ws : ws + w2n],
                        h1i[:, ws : ws + w2n],
                        ns1r[:, nscol : nscol + w2n],
                        ns1i[:, nscol : nscol + w2n],
                    ]
                    pg = ps2.tile([D, 2 * 512], f32, tag="pg")
                    dsts = (pg[:, 0:w2n], pg[:, 512 : 512 + w2n])
                    for comp, ptile in enumerate(dsts):
                        for t_i, rhs in enumerate(rhs_list):
                            wcol = (comp * 4 + t_i) * D
                            nc.tensor.matmul(
                                ptile, wg[:, wcol : wcol + D], rhs,
                                start=(t_i == 0), stop=(t_i == 3),
                            )

                    # wide relu over both psum banks (path A), or only over the
                    # scan component's bank (path B; the d-comp relu is fused
                    # into the final STT from PSUM)
                    h2 = st2.tile([D, 2 * 512], f16, tag="h2", bufs=6)
                    wlo = 512 if wcomp_is_i else 0
                    dlo = 0 if wcomp_is_i else 512
                    if path_b:
                        nc.scalar.activation(
                            h2[:, wlo : wlo + w2n], pg[:, wlo : wlo + w2n],
                            AF.Relu, bias=bias[:, 5:6],
                        )
                    else:
                        nc.scalar.activation(
                            h2[:, 0 : 512 + w2n], pg[:, 0 : 512 + w2n], AF.Relu,
                            bias=bias[:, 5:6],
                        )
                    h2w = h2[:, wlo : wlo + 512]
                    h2d = h2[:, dlo : dlo + 512]

                    wt = st2.tile([D, CHUNK], f16, tag="wt", bufs=6)
                    if k == 0:
                        winit = st2.tile([D, 1], f32, tag="winit")
                        nc.vector.tensor_reduce(
                            winit[:, 0:1], h2w[:, 1:18], AX.X, OP.add
                        )
                        iw = winit[:, 0:1]
                    else:
                        iw = state["prev_wt"][:, state["prev_cw"] - 1 : state["prev_cw"]]
                    nc.vector.tensor_tensor_scan(
                        wt[:, :cw], h2w[:, 18 : 18 + cw], h2w[:, 1 : 1 + cw],
                        iw, OP.add, OP.subtract,
                    )
                    state["prev_wt"], state["prev_cw"] = wt, cw
                    if k == 1:
                        dbg_dump(4, h2[:, 0:512])
                        dbg_dump(5, h2[:, 512:1024])
                        dbg_dump(6, wt[:, 0:492], w=492)

                    state["a"][k] = (wt, h2w, h2d, pg, path_b, cw)

                def do_stage2b(k):
                    # d' = sm*wt + sw*h2w ; out' = d' + h2d (or STT-relu fused)
                    wt, h2w, h2d, pg, path_b, cw = state["a"].pop(k)
                    dlo = 0 if wcomp_is_i else 512
                    eng_d = nc.gpsimd if (gp_d and tt_d) else nc.vector
                    t1 = st2.tile([D, CHUNK], f16, tag="t1", bufs=4)
                    if tt_d:
                        if sm > 0 and sw < 0:
                            eng_d.tensor_tensor(
                                t1[:, :cw], wt[:, :cw], h2w[:, 10 : 10 + cw],
                                OP.subtract,
                            )
                        elif sm > 0 and sw > 0:
                            eng_d.tensor_tensor(
                                t1[:, :cw], wt[:, :cw], h2w[:, 10 : 10 + cw],
                                OP.add,
                            )
                        else:  # sm < 0: t1 = h2w*sw' - wt handled by STT below
                            nc.vector.scalar_tensor_tensor(
                                t1[:, :cw], wt[:, :cw], sm,
                                h2w[:, 10 : 10 + cw], OP.mult,
                                OP.add if sw > 0 else OP.subtract,
                            )
                    else:
                        nc.vector.scalar_tensor_tensor(
                            t1[:, :cw], wt[:, :cw], sm, h2w[:, 10 : 10 + cw],
                            OP.mult, OP.add if sw > 0 else OP.subtract,
                        )
                    ot = outp.tile([D, CHUNK], f16)
                    if path_b:
                        nc.vector.scalar_tensor_tensor(
                            ot[:, :cw], pg[:, dlo + 10 : dlo + 10 + cw], 0.0,
                            t1[:, :cw], OP.max, OP.add,
                        )
                    else:
                        eng_o = nc.gpsimd if gp_out else nc.vector
                        eng_o.tensor_tensor(
                            ot[:, :cw], t1[:, :cw], h2d[:, 10 : 10 + cw], OP.add
                        )
                    if k == 1:
                        dbg_dump(7, t1[:, 0:492], w=492)
                    nc.sync.dma_start(
                        outT[:, k * CHUNK : k * CHUNK + cw], ot[:, :cw]
                    )

                n1 = (LH + L1_PAIR - 1) // L1_PAIR
                nw = (NS - 2 + PW - 1) // PW
                n2 = (L + CHUNK - 1) // CHUNK
                for t in range(max(n1, nw + 1, n2 // 2 + 4)):
                    if t < n1:
                        do_l1(t)
                    if t == 0:
                        do_win_init()
                    for j in (2 * (t - 2), 2 * (t - 2) + 1):
                        if 0 <= j - 2 < n2:
                            do_stage2b(j - 2)
                    for j in (2 * (t - 2), 2 * (t - 2) + 1):
                        if 0 <= j < n2:
                            do_stage2a(j)
                    if 0 <= t - 1 < nw:
                        do_win(t - 1)

    nc.compile()
    return nc


def _build_program_v2(reps=1):
    """fp16 pipeline: fused diff+scan window sums, gamma-folded layer-2 bias,
    merged wide activations, 3-deep software pipeline across engines."""
    import concourse.bacc as bacc
    import concourse.bass as bass
    import concourse.mybir as mybir
    import concourse.tile as tile

    f32 = mybir.dt.float32
    f16 = mybir.dt.float16
    AF = mybir.ActivationFunctionType
    OP = mybir.AluOpType
    AX = mybir.AxisListType
    sc = _build_program.scalars
    use_gamma = bool(sc.get("use_gamma"))
    drop_h2c = abs(sc["h2c_scale"]) < 0.01

    nc = bacc.Bacc("TRN2", target_bir_lowering=False, debug=False)

    xT = nc.dram_tensor("xT", [D, LH], f16, kind="ExternalInput")
    wl1_d = nc.dram_tensor("wl1", [D, 2 * D], f16, kind="ExternalInput")
    wg_d = nc.dram_tensor("wg", [D, 8 * D], f16, kind="ExternalInput")
    bias_d = nc.dram_tensor("biases", [D, 8], f32, kind="ExternalInput")
    outT = nc.dram_tensor("outT", [D, L], f16, kind="ExternalOutput")

    with tile.TileContext(nc) as tc:
        with (
            tc.tile_pool(name="consts", bufs=1) as cpool,
            tc.tile_pool(name="slab", bufs=1) as slab,
            tc.tile_pool(name="xs", bufs=3) as xs,
            tc.tile_pool(name="ps1", bufs=1, space="PSUM") as ps1,
            tc.tile_pool(name="ps2", bufs=2, space="PSUM") as ps2,
            tc.tile_pool(name="st2", bufs=4) as st2,
            tc.tile_pool(name="outp", bufs=4) as outp,
        ):
            wl1 = cpool.tile([D, 2 * D], f16)
            wg = cpool.tile([D, 8 * D], f16)
            bias = cpool.tile([D, 8], f32)
            nc.sync.dma_start(wl1[:], wl1_d[:])
            nc.sync.dma_start(wg[:], wg_d[:])
            nc.sync.dma_start(bias[:], bias_d[:])

            for _rep in range(reps):
                h1r = slab.tile([D, LH], f16, tag="h1r", bufs=2)
                h1i = slab.tile([D, LH], f16, tag="h1i", bufs=2)
                ns1r = slab.tile([D, NS], f16, tag="ns1r")
                ns1i = slab.tile([D, NS], f16, tag="ns1i")
                init1 = slab.tile([D, 4], f32, tag="init1")
                init2 = slab.tile([D, 4], f32, tag="init2")
                state = dict(prev_wt=None, prev_cw=0, a={}, b={})

                def do_l1(k):
                    # 1024-wide pair: 2 matmuls per component into a 2-bank
                    # PSUM tile, ONE wide ACT per component
                    s = k * L1_PAIR
                    cw2 = min(L1_PAIR, LH - s)
                    xt = xs.tile([D, L1_PAIR], f16)
                    nc.sync.dma_start(xt[:, :cw2], xT[:, s : s + cw2])
                    pr = ps1.tile([D, L1_PAIR], f32, tag="ps1r")
                    pi = ps1.tile([D, L1_PAIR], f32, tag="ps1i")
                    for h0 in range(0, cw2, L1_CHUNK):
                        hw = min(L1_CHUNK, cw2 - h0)
                        nc.tensor.matmul(
                            pr[:, h0 : h0 + hw], wl1[:, 0:D], xt[:, h0 : h0 + hw],
                            start=True, stop=True,
                        )
                        nc.tensor.matmul(
                            pi[:, h0 : h0 + hw], wl1[:, D : 2 * D],
                            xt[:, h0 : h0 + hw], start=True, stop=True,
                        )
                    nc.scalar.activation(
                        h1r[:, s : s + cw2], pr[:, :cw2], AF.Relu, bias=bias[:, 0:1]
                    )
                    nc.scalar.activation(
                        h1i[:, s : s + cw2], pi[:, :cw2], AF.Relu, bias=bias[:, 1:2]
                    )

                def do_win_init():
                    # ns1 col j <-> center c = HL-10+j; cols 0/1 via direct
                    # reduces so scan pieces start 4B-aligned. With use_gamma
                    # the scan state carries +gamma (folds b2 through GEMM2).
                    nc.vector.tensor_reduce(init1[:, 0:1], h1r[:, 0:17], AX.X, OP.add)
                    nc.vector.tensor_reduce(init1[:, 1:2], h1i[:, 0:17], AX.X, OP.add)
                    nc.vector.tensor_reduce(init1[:, 2:3], h1r[:, 1:18], AX.X, OP.add)
                    nc.vector.tensor_reduce(init1[:, 3:4], h1i[:, 1:18], AX.X, OP.add)
                    if use_gamma:
                        for c, gcol in ((0, 2), (1, 3), (2, 2), (3, 3)):
                            nc.vector.tensor_scalar(
                                init2[:, c : c + 1], init1[:, c : c + 1], 1.0,
                                bias[:, gcol : gcol + 1], OP.mult, OP.add,
                            )
                    else:
                        nc.vector.tensor_copy(init2[:], init1[:])
                    nc.vector.tensor_copy(ns1r[:, 0:1], init2[:, 0:1])
                    nc.vector.tensor_copy(ns1i[:, 0:1], init2[:, 1:2])
                    nc.vector.tensor_copy(ns1r[:, 1:2], init2[:, 2:3])
                    nc.vector.tensor_copy(ns1i[:, 1:2], init2[:, 3:4])

                def do_win(p):
                    # fused diff+scan: state = (h1[c+8] add state) sub h1[c-9]
                    s = 2 + p * PW
                    w = min(PW, NS - s)
                    ir = init2[:, 2:3] if p == 0 else ns1r[:, s - 1 : s]
                    ii = init2[:, 3:4] if p == 0 else ns1i[:, s - 1 : s]
                    nc.vector.tensor_tensor_scan(
                        ns1r[:, s : s + w], h1r[:, s + 16 : s + 16 + w],
                        h1r[:, s - 1 : s - 1 + w], ir, OP.add, OP.subtract,
                    )
                    nc.vector.tensor_tensor_scan(
                        ns1i[:, s : s + w], h1i[:, s + 16 : s + 16 + w],
                        h1i[:, s - 1 : s - 1 + w], ii, OP.add, OP.subtract,
                    )

                def do_stage2a(k):
                    a = HL + k * CHUNK
                    cw = min(CHUNK, L - k * CHUNK)
                    ws = a - 10           # h1 slab col of h2-window col 0 (even)
                    w2n = cw + 18
                    nscol = k * CHUNK     # ns1 col of center ws

                    rhs_list = [
                        h1r[:, ws : ws + w2n],
                        h1i[:, ws : ws + w2n],
                        ns1r[:, nscol : nscol + w2n],
                        ns1i[:, nscol : nscol + w2n],
                    ]
                    if use_gamma:
                        pg = ps2.tile([D, 2 * 512], f32, tag="pg")
                        dsts = (pg[:, 0:w2n], pg[:, 512 : 512 + w2n])
                    else:
                        pgr = ps2.tile([D, CHUNK + 18], f32, tag="ps2r")
                        pgi = ps2.tile([D, CHUNK + 18], f32, tag="ps2i")
                        dsts = (pgr[:, :w2n], pgi[:, :w2n])
                    for comp, ptile in enumerate(dsts):
                        for t_i, rhs in enumerate(rhs_list):
                            wcol = (comp * 4 + t_i) * D
                            nc.tensor.matmul(
                                ptile, wg[:, wcol : wcol + D], rhs,
                                start=(t_i == 0), stop=(t_i == 3),
                            )

                    if use_gamma:
                        h2 = st2.tile([D, 2 * 512], f16, tag="h2", bufs=6)
                        nc.scalar.activation(
                            h2[:, 0 : 512 + w2n], pg[:, 0 : 512 + w2n], AF.Relu,
                            bias=bias[:, 5:6],
                        )
                        h2r = h2[:, 0:512]
                        h2i = h2[:, 512 : 2 * 512]
                    else:
                        h2r = st2.tile([D, CHUNK + 18], f16, tag="h2r", bufs=6)
                        h2i = st2.tile([D, CHUNK + 18], f16, tag="h2i", bufs=6)
                        nc.scalar.activation(
                            h2r[:, :w2n], dsts[0], AF.Relu, bias=bias[:, 2:3]
                        )
                        nc.scalar.activation(
                            h2i[:, :w2n], dsts[1], AF.Relu, bias=bias[:, 3:4]
                        )

                    if not drop_h2c:
                        h2c = st2.tile([D, CHUNK + 18], f16, tag="h2c")
                        if sc["h2c_on_r"]:
                            nc.vector.scalar_tensor_tensor(
                                h2c[:, :w2n], h2r[:, :w2n], sc["h2c_scale"],
                                h2i[:, :w2n], OP.mult, OP.add,
                            )
                        else:
                            nc.vector.scalar_tensor_tensor(
                                h2c[:, :w2n], h2i[:, :w2n], sc["h2c_scale"],
                                h2r[:, :w2n], OP.mult, OP.add,
                            )
                        h2cv = h2c
                    else:
                        h2cv = h2i if sc["h2c_on_r"] else h2r
                    # wt = chained fused diff+scan -> W17(h2c) per out column
                    wt = st2.tile([D, CHUNK], f16, tag="wt", bufs=6)
                    if k == 0:
                        winit = st2.tile([D, 1], f32, tag="winit")
                        nc.vector.tensor_reduce(
                            winit[:, 0:1], h2cv[:, 1:18], AX.X, OP.add
                        )
                        iw = winit[:, 0:1]
                    else:
                        iw = state["prev_wt"][:, state["prev_cw"] - 1 : state["prev_cw"]]
                    nc.vector.tensor_tensor_scan(
                        wt[:, :cw], h2cv[:, 18 : 18 + cw], h2cv[:, 1 : 1 + cw],
                        iw, OP.add, OP.subtract,
                    )
                    state["prev_wt"], state["prev_cw"] = wt, cw

                    state["a"][k] = (wt, h2r, h2i, cw)

                def do_stage2b(k):
                    # out = c0*(h2r + r1*h2i + qn*wt) + eb2r; pure-DVE STT
                    # chain, emitted one step late so inputs are already done
                    wt, h2r, h2i, cw = state["a"].pop(k)
                    t1 = st2.tile([D, CHUNK], f16, tag="t1", bufs=4)
                    nc.vector.scalar_tensor_tensor(
                        t1[:, :cw], h2i[:, 10 : 10 + cw], sc["r1"],
                        h2r[:, 10 : 10 + cw], OP.mult, OP.add,
                    )
                    t2 = st2.tile([D, CHUNK], f16, tag="t2", bufs=4)
                    nc.vector.scalar_tensor_tensor(
                        t2[:, :cw], wt[:, :cw], sc["qn"], t1[:, :cw],
                        OP.mult, OP.add,
                    )
                    ot = outp.tile([D, CHUNK], f16)
                    nc.vector.tensor_scalar(
                        ot[:, :cw], t2[:, :cw], sc["c0"], bias[:, 4:5],
                        OP.mult, OP.add,
                    )
                    nc.sync.dma_start(
                        outT[:, k * CHUNK : k * CHUNK + cw], ot[:, :cw]
                    )

                # interleaved pipeline, ~1024 cols per step on every lane:
                # layer-1 pair t | window piece t-1 | stage2 a/b/c staggered
                n1 = (LH + L1_PAIR - 1) // L1_PAIR
                nw = (NS - 2 + PW - 1) // PW
                n2 = (L + CHUNK - 1) // CHUNK
                # emission order: aged b/c sub-stages first (their inputs
                # are 1-2 steps old, so the DVE head never stalls on them),
                # fresh a/win work last
                for t in range(max(n1, nw + 1, n2 // 2 + 4)):
                    if t < n1:
                        do_l1(t)
                    if t == 0:
                        do_win_init()
                    for j in (2 * (t - 2), 2 * (t - 2) + 1):
                        if 0 <= j - 2 < n2:
                            do_stage2b(j - 2)
                    for j in (2 * (t - 2), 2 * (t - 2) + 1):
                        if 0 <= j < n2:
                            do_stage2a(j)
                    if 0 <= t - 1 < nw:
                        do_win(t - 1)

    nc.compile()
    return nc


def _build_program_v1(matmul_dtype="float32r", reps=1):
    """fp32 baseline (prefix-scan + shifted-difference, all-DVE elementwise)."""
    import concourse.bacc as bacc
    import concourse.bass as bass
    import concourse.mybir as mybir
    import concourse.tile as tile

    f32 = mybir.dt.float32
    mm_dt = getattr(mybir.dt, matmul_dtype)
    AF = mybir.ActivationFunctionType
    OP = mybir.AluOpType
    V1_HL = 18
    V1_LH = L + 36

    nc = bacc.Bacc("TRN2", target_bir_lowering=False, debug=False)

    xT = nc.dram_tensor("xT", [D, V1_LH], mm_dt, kind="ExternalInput")
    wl1_d = nc.dram_tensor("wl1", [D, 2 * D], mm_dt, kind="ExternalInput")
    wg_d = nc.dram_tensor("wg", [D, 8 * D], mm_dt, kind="ExternalInput")
    bias_d = nc.dram_tensor("biases", [D, 8], f32, kind="ExternalInput")
    outT = nc.dram_tensor("outT", [D, L], f32, kind="ExternalOutput")

    with tile.TileContext(nc) as tc:
        with (
            tc.tile_pool(name="consts", bufs=1) as cpool,
            tc.tile_pool(name="slab", bufs=1) as slab,
            tc.tile_pool(name="xs", bufs=3) as xs,
            tc.tile_pool(name="ps1", bufs=2, space="PSUM") as ps1,
            tc.tile_pool(name="ps2", bufs=2, space="PSUM") as ps2,
            tc.tile_pool(name="st2", bufs=4) as st2,
            tc.tile_pool(name="outp", bufs=3) as outp,
        ):
            wl1 = cpool.tile([D, 2 * D], mm_dt)
            wg = cpool.tile([D, 8 * D], mm_dt)
            bias = cpool.tile([D, 8], f32)
            nc.sync.dma_start(wl1[:], wl1_d[:])
            nc.sync.dma_start(wg[:], wg_d[:])
            nc.sync.dma_start(bias[:], bias_d[:])

            for _rep in range(reps):
                h1r = slab.tile([D, V1_LH], mm_dt, tag="h1r")
                h1i = slab.tile([D, V1_LH], mm_dt, tag="h1i")

                n1 = (V1_LH + L1_CHUNK - 1) // L1_CHUNK
                for k in range(n1):
                    s = k * L1_CHUNK
                    cw = min(L1_CHUNK, V1_LH - s)
                    xt = xs.tile([D, L1_CHUNK], mm_dt)
                    nc.sync.dma_start(xt[:, :cw], xT[:, s : s + cw])
                    pr = ps1.tile([D, L1_CHUNK], f32, tag="ps1r")
                    pi = ps1.tile([D, L1_CHUNK], f32, tag="ps1i")
                    nc.tensor.matmul(
                        pr[:, :cw], wl1[:, 0:D], xt[:, :cw], start=True, stop=True
                    )
                    nc.tensor.matmul(
                        pi[:, :cw], wl1[:, D : 2 * D], xt[:, :cw],
                        start=True, stop=True,
                    )
                    nc.scalar.activation(
                        h1r[:, s : s + cw], pr[:, :cw], AF.Relu, bias=bias[:, 0:1]
                    )
                    nc.scalar.activation(
                        h1i[:, s : s + cw], pi[:, :cw], AF.Relu, bias=bias[:, 1:2]
                    )

                n2 = (L + CHUNK - 1) // CHUNK
                for k in range(n2):
                    a = V1_HL + k * CHUNK
                    cw = min(CHUNK, L - k * CHUNK)
                    w2n = cw + 17 + ((cw + 17) % 2)
                    w1s, w1n = a - 18, w2n + 17
                    w2s = a - 9

                    dve_view = (
                        (lambda ap: ap.bitcast(f32))
                        if matmul_dtype == "float32r"
                        else (lambda ap: ap)
                    )
                    p1r = st2.tile([D, CHUNK + 36], f32, tag="p1r")
                    p1i = st2.tile([D, CHUNK + 36], f32, tag="p1i")
                    nc.vector.tensor_tensor_scan(
                        p1r[:, :w1n], dve_view(h1r[:, w1s : w1s + w1n]),
                        dve_view(h1r[:, w1s : w1s + w1n]), 0.0, OP.add, OP.bypass,
                    )
                    nc.vector.tensor_tensor_scan(
                        p1i[:, :w1n], dve_view(h1i[:, w1s : w1s + w1n]),
                        dve_view(h1i[:, w1s : w1s + w1n]), 0.0, OP.add, OP.bypass,
                    )
                    ns1r = st2.tile([D, CHUNK + 18], mm_dt, tag="ns1r")
                    ns1i = st2.tile([D, CHUNK + 18], mm_dt, tag="ns1i")
                    nc.vector.scalar_tensor_tensor(
                        ns1r[:, :w2n], p1r[:, 17 : 17 + w2n], 1.0, p1r[:, 0:w2n],
                        OP.mult, OP.subtract,
                    )
                    nc.vector.scalar_tensor_tensor(
                        ns1i[:, :w2n], p1i[:, 17 : 17 + w2n], 1.0, p1i[:, 0:w2n],
                        OP.mult, OP.subtract,
                    )

                    pgr = ps2.tile([D, CHUNK + 18], f32, tag="ps2r")
                    pgi = ps2.tile([D, CHUNK + 18], f32, tag="ps2i")
                    rhs_list = [
                        h1r[:, w2s : w2s + w2n],
                        h1i[:, w2s : w2s + w2n],
                        ns1r[:, :w2n],
                        ns1i[:, :w2n],
                    ]
                    for comp, ptile in ((0, pgr), (1, pgi)):
                        for t_i, rhs in enumerate(rhs_list):
                            wcol = (comp * 4 + t_i) * D
                            nc.tensor.matmul(
                                ptile[:, :w2n], wg[:, wcol : wcol + D], rhs,
                                start=(t_i == 0), stop=(t_i == 3),
                            )

                    h2r = st2.tile([D, CHUNK + 18], f32, tag="h2r")
                    h2i = st2.tile([D, CHUNK + 18], f32, tag="h2i")
                    nc.scalar.activation(
                        h2r[:, :w2n], pgr[:, :w2n], AF.Relu, bias=bias[:, 2:3]
                    )
                    nc.scalar.activation(
                        h2i[:, :w2n], pgi[:, :w2n], AF.Relu, bias=bias[:, 3:4]
                    )

                    sc = _build_program.scalars
                    h2c = st2.tile([D, CHUNK + 18], f32, tag="h2c")
                    if sc["h2c_on_r"]:
                        nc.vector.scalar_tensor_tensor(
                            h2c[:, :w2n], h2r[:, :w2n], sc["h2c_scale"],
                            h2i[:, :w2n], OP.mult, OP.add,
                        )
                    else:
                        nc.vector.scalar_tensor_tensor(
                            h2c[:, :w2n], h2i[:, :w2n], sc["h2c_scale"],
                            h2r[:, :w2n], OP.mult, OP.add,
                        )
                    u = st2.tile([D, CHUNK + 18], f32, tag="u")
                    nc.vector.tensor_tensor_scan(
                        u[:, :w2n], h2c[:, :w2n], h2c[:, :w2n], 0.0,
                        OP.add, OP.bypass,
                    )
                    t1 = st2.tile([D, CHUNK], f32, tag="t1")
                    t2 = st2.tile([D, CHUNK], f32, tag="t2")
                    t3 = st2.tile([D, CHUNK], f32, tag="t3")
                    ot = outp.tile([D, CHUNK], f32)
                    nc.vector.scalar_tensor_tensor(
                        t1[:, :cw], h2i[:, 9 : 9 + cw], sc["r1"],
                        h2r[:, 9 : 9 + cw], OP.mult, OP.add,
                    )
                    nc.vector.scalar_tensor_tensor(
                        t2[:, :cw], u[:, 17 : 17 + cw], sc["qn"], t1[:, :cw],
                        OP.mult, OP.add,
                    )
                    nc.vector.scalar_tensor_tensor(
                        t3[:, :cw], u[:, 0:cw], -sc["qn"], t2[:, :cw],
                        OP.mult, OP.add,
                    )
                    nc.vector.tensor_scalar(
                        ot[:, :cw], t3[:, :cw], sc["c0"], bias[:, 4:5],
                        OP.mult, OP.add,
                    )
                    nc.sync.dma_start(
                        outT[:, k * CHUNK : k * CHUNK + cw], ot[:, :cw]
                    )

    nc.compile()
    return nc


def _build_program(mm_dtype="float16", reps=1):
    if _VARIANT == "v3":
        return _build_program_v3(reps=reps)
    if _VARIANT == "v2":
        return _build_program_v2(reps=reps)
    return _build_program_v1("float32r" if mm_dtype == "float16" else mm_dtype, reps)


_MM_DTYPE = "float16"


def _get_program(scalars):
    global _PROGRAM
    _build_program.scalars = scalars
    if _PROGRAM is None:
        _PROGRAM = _build_program(_MM_DTYPE)
    return _PROGRAM


def _reset_program(variant):
    global _VARIANT, _PROGRAM, _EXEC, _MM_DTYPE
    _VARIANT = variant
    if variant == "v1":
        _MM_DTYPE = "float32r"
    _PROGRAM = None
    _EXEC = None


# ------------------------------------------------------- cached PJRT runner
_EXEC = None  # (sharded_fn, in_names, out_names, out_avals, n_params)


def _get_executable(nc):
    """Build (once) a jitted shard_map executable for the 8-core SPMD run,
    mirroring concourse.bass2jax.run_bass_via_pjrt but cached so repeat
    calls don't re-trace/re-compile."""
    global _EXEC
    if _EXEC is not None:
        return _EXEC
    import jax
    import numpy as _np
    from jax.sharding import Mesh, PartitionSpec
    from jax.experimental.shard_map import shard_map

    import concourse.mybir as mybir
    from concourse import bass2jax

    bass2jax.install_neuronx_cc_hook()

    partition_name = (
        nc.partition_id_tensor.name if nc.partition_id_tensor else None
    )
    in_names, out_names, out_avals = [], [], []
    for alloc in nc.m.functions[0].allocations:
        if not isinstance(alloc, mybir.MemoryLocationSet):
            continue
        name = alloc.memorylocations[0].name
        if alloc.kind == "ExternalInput":
            if name != partition_name:
                in_names.append(name)
        elif alloc.kind == "ExternalOutput":
            out_names.append(name)
            out_avals.append(
                jax.core.ShapedArray(
                    tuple(alloc.tensor_shape), mybir.dt.np(alloc.dtype)
                )
            )
    n_params = len(in_names)
    all_names = in_names + out_names
    if partition_name is not None:
        all_names = all_names + [partition_name]

    def _body(*args):
        operands = list(args)
        if partition_name is not None:
            operands.append(bass2jax.partition_id_tensor())
        outs = bass2jax._bass_exec_p.bind(
            *operands,
            out_avals=tuple(out_avals),
            in_names=tuple(all_names),
            out_names=tuple(out_names),
            lowering_input_output_aliases=(),
            sim_require_finite=True,
            sim_require_nnan=True,
            nc=nc,
        )
        return tuple(outs)

    devices = jax.devices()[:NCORES]
    mesh = Mesh(_np.asarray(devices), ("core",))
    in_specs = (PartitionSpec("core"),) * (n_params + len(out_names))
    out_specs = (PartitionSpec("core"),) * len(out_names)
    donate = tuple(range(n_params, n_params + len(out_names)))
    sharded = jax.jit(
        shard_map(
            _body, mesh=mesh, in_specs=in_specs, out_specs=out_specs, check_rep=False
        ),
        donate_argnums=donate,
        keep_unused=True,
    )
    _EXEC = (sharded, in_names, out_names, out_avals, n_params)
    return _EXEC


def _execute(in_maps):
    import jax.numpy as jnp

    nc = _PROGRAM
    sharded, in_names, out_names, out_avals, n_params = _get_executable(nc)
    concat_in = [
        np.concatenate([m[name] for m in in_maps], axis=0) for name in in_names
    ]
    zeros = [
        jnp.zeros((NCORES * a.shape[0], *a.shape[1:]), a.dtype) for a in out_avals
    ]
    out_arrs = sharded(*concat_in, *zeros)
    return {
        name: np.asarray(out_arrs[i]).reshape(NCORES, *out_avals[i].shape)
        for i, name in enumerate(out_names)
    }


# ---------------------------------------------------------------- entrypoint
def _prepare(ins):
    global _VARIANT
    ins = {k: np.asarray(v) for k, v in ins.items()}
    w1 = _evolution_row(DEG, float(ins["t1r"]), float(ins["t1i"]))
    w2 = _evolution_row(DEG, float(ins["t2r"]), float(ins["t2i"]))
    if _VARIANT == "v3":
        wl1, wg, biases, scalars = _fold_weights_v3(ins, w1, w2)
        if not scalars.get("v3_ok"):
            _reset_program("v2")
            wl1, wg, biases, scalars = _fold_weights(ins, w1, w2)
    else:
        wl1, wg, biases, scalars = _fold_weights(ins, w1, w2)
    _POST["sigma"] = float(scalars.get("v3_sigma", 1.0)) if _VARIANT == "v3" else 1.0
    _POST["eb2r"] = (
        ins["eb2r"].astype(np.float32) if _VARIANT == "v3" else None
    )
    _get_program(scalars)

    dt = np.float32 if _VARIANT == "v1" else np.float16
    x = ins["x"].astype(dt, copy=False)
    idx = np.arange(-HL, L + HR)
    in_maps = []
    for c in range(NCORES):
        rows = (c * L + idx) % N
        xTslab = np.ascontiguousarray(x[rows].T)
        in_maps.append(
            {
                "xT": xTslab,
                "wl1": wl1.astype(dt),
                "wg": wg.astype(dt),
                "biases": biases,
            }
        )
    return in_maps


_POST = {"sigma": 1.0, "eb2r": None}


def _run(ins, trace=False):
    ins = {k: np.asarray(v) for k, v in ins.items()}
    if not _is_circulant(ins["edge_index"]):
        return _fallback_numpy(ins), None
    try:
        in_maps = _prepare(ins)
        outs = _execute(in_maps)
    except Exception:
        if _VARIANT == "v1":
            raise
        _reset_program("v2" if _VARIANT == "v3" else "v1")
        try:
            in_maps = _prepare(ins)
            outs = _execute(in_maps)
        except Exception:
            if _VARIANT == "v1":
                raise
            _reset_program("v1")
            in_maps = _prepare(ins)
            outs = _execute(in_maps)
    sigma, eb2r = _POST["sigma"], _POST["eb2r"]
    out = np.empty((N, D), np.float32)
    for c in range(NCORES):
        blk = outs["outT"][c].T.astype(np.float32)
        if sigma != 1.0:
            blk = blk * sigma
        if eb2r is not None and np.any(eb2r):
            blk = blk + eb2r
        out[c * L : (c + 1) * L] = blk
    return out, None


def kernel(**inputs):
    out, _ = _run(inputs)
    return out

